# revision 30
# baseline (speedup 1.0000x reference)
"""Trainium2 Bass kernel for AttentionLateralOp.

Reference computation (per batch b):
    x = origin_out[b].reshape(C, N)      # keys/values source
    t = target_in[b].reshape(C, N)       # queries source + residual
    f = Wq @ t          [CQK, N]
    g = Wk @ x          [CQK, N]
    v = Wv @ x          [C, N]
    scores = f^T @ g    [N, N]
    beta = softmax(scores, axis=0)       # over i (rows)
    o = gamma * v @ beta + t

Sharding: 8 cores = (batch b = core//2) x (half of the j/output axis =
core%2). Each core computes the full f and v^T for its batch, and the
j-shard of g / scores / output.

Pipeline layout (v3): one fused stream ordered by data arrival.
 - DMA is striped round-robin across the three hardware queues
   (sync/scalar/gpsimd) in 0.5MB units sorted by deadline, so each
   tensor lands roughly when its consumer needs it.
 - The in-order PE queue is emitted to chase the stream: f chunks as t
   arrives, g + subsampled row-max per x j-shard group, v^T chunks as
   soon as Wv and x land, then score-tile 0 group-by-group (group g
   needs only f chunk g), with the o-accumulation of tile 0 chasing the
   exp of tile 0 while the tail of f / v^T / score-tile-1 is woven in.
 - Steady state: o-accumulation for tile k with the score matmuls of
   tile k+1 woven in every 8 i-steps. PSUM: 2x2-bank score buffers +
   2x2-bank output accumulators = 8 banks.
 - E and v^T are held in bf16 (halves SBUF, same PE rate); exp runs on
   Scalar in 1024-element groups chasing the score matmuls.

Softmax-over-the-contraction-axis trick: append a ones row to f and a
(-mhat_j) row to g, so the PE emits max-subtracted logits directly into
PSUM; Z_j comes from a ones column appended to v^T; the final gamma/Z_j
scaling and +t residual are per-partition ops in the transposed [j, c]
output orientation (output is transposed back on the host).
"""

import os
import sys

for _p in ("/opt/trn_rl_repo", "/root/.axon_site/_ro/trn_rl_repo"):
    if os.path.isdir(_p):
        sys.path.insert(0, _p)
        break

import ml_dtypes
import numpy as np

import concourse.bass as bass  # noqa: F401  (bass types via bacc)
import concourse.tile as tile
from concourse import bacc, mybir
from concourse.bass import ds, ts
from concourse.bass_utils import run_bass_kernel_spmd
from concourse.masks import make_identity

F32 = mybir.dt.float32
F32R = mybir.dt.float32r
F16 = mybir.dt.float16
BF16 = mybir.dt.bfloat16
AF = mybir.ActivationFunctionType
ALU = mybir.AluOpType
AX = mybir.AxisListType

B, C, H, W = 4, 512, 64, 64
N = H * W            # 4096
CQK = C // 8         # 64
NCORES = 8
NJ = B * N // NCORES  # 2048 columns of the j axis per core
JT = 256             # j-tile width of a score tile
NJT = NJ // JT       # 8 score tiles
NIC = N // 128       # 32 i-chunks
NCC = C // 128       # 4 contraction chunks over C
EG = 2               # i-chunks per exp group (psc tile = 1 PSUM bank)
NEG = NIC // EG      # 8 exp groups per score tile


def _build():
    nc = bacc.Bacc(None, target_bir_lowering=False)

    x_d = nc.dram_tensor(
        "x", [NIC // 8, 128, NCC * 4 * 128], F16, kind="ExternalInput"
    )
    xb_d = nc.dram_tensor(
        "xb", [NIC // 8, 128, NCC * 4 * 128], BF16, kind="ExternalInput"
    )
    t_d = nc.dram_tensor("t", [C, N], F16, kind="ExternalInput")
    ttr_d = nc.dram_tensor("ttr", [NJ, C], F32, kind="ExternalInput")
    wqt_d = nc.dram_tensor("wqt", [C, CQK], F16, kind="ExternalInput")
    wkt_d = nc.dram_tensor("wkt", [C, CQK], F16, kind="ExternalInput")
    wvt_d = nc.dram_tensor("wvt", [C, C], F16, kind="ExternalInput")
    gam_d = nc.dram_tensor("gam", [128, 1], F32, kind="ExternalInput")
    o_d = nc.dram_tensor("o", [NJ, C], F32, kind="ExternalOutput")

    with tile.TileContext(nc) as tc:
        with (
            tc.tile_pool(name="persist", bufs=1) as persist,
            tc.tile_pool(name="wpool", bufs=1) as wpool,
            tc.tile_pool(name="tstream", bufs=8) as tstream,
            tc.tile_pool(name="xfpool", bufs=4) as xfpool,
            tc.tile_pool(name="xfbpool", bufs=4) as xfbpool,
            tc.tile_pool(name="epool", bufs=3) as epool,
            tc.tile_pool(name="ttrp", bufs=2) as ttrp,
            tc.tile_pool(name="obp", bufs=2) as obp,
            tc.tile_pool(name="zp", bufs=2) as zp,
            tc.tile_pool(name="pssc", bufs=2, space="PSUM") as pssc,
        ):
            # ---- persistent SBUF ----
            vt = persist.tile([128, NIC, C + 2], BF16)
            fp = persist.tile([CQK + 1, N], F32R)
            gp = persist.tile([CQK + 1, NJ], F32R)
            ident = persist.tile([128, 128], F32)
            mall = persist.tile([128, 16], F32)
            nmneg = persist.tile([4, 16 * 128], F32R)
            gam_sb = persist.tile([128, 1], F32)

            wqt_sb = wpool.tile([128, NCC, CQK], F16)
            wkt_sb = wpool.tile([128, NCC, CQK], F16)
            wvt_sb = wpool.tile([128, NCC, C], F16)
            wvb_sb = wpool.tile([128, NCC, C], BF16)
            warm_sb = persist.tile([128, 256], F32R)

            # ---- DMA: build the deadline-ordered unit list, stripe RR ----
            t_tiles = {
                it: tstream.tile([128, NCC, 512], F16, name="tt")
                for it in range(N // 512)
            }
            xf_tiles = {
                g4: xfpool.tile([128, NCC, 4, 128], F16, name="xf")
                for g4 in range(4)
            }
            xfb_tiles = {
                g4: xfbpool.tile([128, NCC, 4, 128], BF16, name="xfb")
                for g4 in range(4, 8)
            }

            def u_t(it):
                return lambda eng: eng.dma_start(
                    t_tiles[it],
                    t_d[:, ts(it, 512)].rearrange("(cc p) n -> p cc n", cc=NCC),
                )

            def u_x(g4):
                return lambda eng: eng.dma_start(xf_tiles[g4], x_d[g4])

            def u_xb(g4):  # one bf16 x group (v-only tail), 0.5MB
                return lambda eng: eng.dma_start(
                    xfb_tiles[g4], xb_d[g4 - 4]
                )

            def u_wq(eng):
                eng.dma_start(
                    wqt_sb, wqt_d.rearrange("(cc p) k -> p cc k", cc=NCC)
                )

            def u_wk(eng):
                eng.dma_start(
                    wkt_sb, wkt_d.rearrange("(cc p) k -> p cc k", cc=NCC)
                )

            def u_wv(eng):
                eng.dma_start(
                    wvt_sb, wvt_d.rearrange("(cc p) e -> p cc e", cc=NCC)
                )

            def u_gam(eng):
                eng.dma_start(gam_sb, gam_d[:])

            units = [u_wq, u_wk, u_t(0), u_wv, u_x(0), u_t(1), u_x(1),
                     u_t(2), u_x(2), u_t(3), u_x(3), u_t(4), u_xb(4),
                     u_t(5), u_xb(5), u_t(6), u_xb(6), u_t(7), u_xb(7),
                     u_gam]
            engs = [nc.sync, nc.scalar, nc.gpsimd]
            # Latin-square dealing: consecutive units AND units 3 apart land
            # on different queues (the unit list has period-3 patterns that
            # would otherwise pin both halves of one tensor to one queue)
            _deal = [0, 1, 2, 1, 2, 0, 2, 0, 1]
            for i, u in enumerate(units):
                u(engs[_deal[i % 9]])

            # constants; warm_sb feeds the HAM warm-up matmuls and must be
            # ready immediately, so its memset rides on vector (no DMA queue)
            nc.vector.memset(warm_sb.bitcast(F32), 0.0)
            make_identity(nc, ident)
            nc.gpsimd.memset(fp[CQK : CQK + 1, :].bitcast(F32), 1.0)
            nc.gpsimd.memset(vt[:, :, C : C + 2], 1.0)
            # bf16 copy of Wv^T for the bf16 v^T tail
            nc.vector.tensor_copy(wvb_sb, wvt_sb)

            # ---- compute emission helpers ----
            psum_pools = {}

            def emit_f(it):
                pf = psum_pools["set"].tile([CQK, 512], F32, tag="ps", name="pf")
                for cc in range(NCC):
                    nc.tensor.matmul(
                        pf,
                        wqt_sb[:, cc, :],
                        t_tiles[it][:, cc, :],
                        start=(cc == 0),
                        stop=(cc == NCC - 1),
                    )
                nc.vector.tensor_copy(fp[0:CQK, ts(it, 512)], pf)

            def emit_v(ic):
                pv = psum_pools["set"].tile([128, C], F32, tag="ps", name="pv")
                g4 = ic // 4
                lhs_t = xf_tiles[g4] if g4 < 4 else xfb_tiles[g4]
                rhs_w = wvt_sb if g4 < 4 else wvb_sb
                for cc in range(NCC):
                    nc.tensor.matmul(
                        pv,
                        lhs_t[:, cc, ic % 4, :],
                        rhs_w[:, cc, :],
                        start=(cc == 0),
                        stop=(cc == NCC - 1),
                    )
                nc.vector.tensor_copy(vt[:, ic, 0:C], pv)

            def emit_warm(n):
                # dummy matmuls: keep the PE busy through DMA waits so the
                # HAM clock gate reaches (and holds) 2.4 GHz from the start.
                # Four accumulating matmuls per PSUM tile -- a single matmul
                # per tile paces at the slot-reuse semaphore latency (~35%
                # duty), which HAM reads as idle.
                for _ in range(n):
                    pw = psum_pools["set"].tile(
                        [128, 256], F32, tag="ps", name="pw"
                    )
                    nc.tensor.matmul(
                        pw, warm_sb[:, 0:128], warm_sb, start=True, stop=True
                    )

            def emit_g(jt4):
                pg = psum_pools["set"].tile([CQK, 512], F32, tag="ps", name="pg")
                for cc in range(NCC):
                    nc.tensor.matmul(
                        pg,
                        wkt_sb[:, cc, :],
                        xf_tiles[jt4][:, cc, :, :],
                        start=(cc == 0),
                        stop=(cc == NCC - 1),
                    )
                nc.vector.tensor_copy(gp[0:CQK, ts(jt4, 512)], pg)

            def emit_p1(jc):
                ps1 = pssc.tile([128, 512], F32, name="psc")
                nc.tensor.matmul(
                    ps1,
                    gp[0:CQK, ts(jc, 128)],
                    fp[0:CQK, 0:512],
                    start=True,
                    stop=True,
                )
                nc.vector.reduce_max(mall[:, jc : jc + 1], ps1, axis=AX.X)

            def emit_mh(g4):
                pmt = psum_pools["set"].tile([16, 128], F32, tag="ps", name="pmt")
                nc.tensor.matmul(
                    pmt[0:4, :], mall[:, 4 * g4 : 4 * g4 + 4], ident,
                    start=True, stop=True,
                )
                nc.scalar.mul(
                    nmneg[0:4, ds(g4 * 128, 128)], pmt[0:4, :], -1.0
                )
                for k in range(4):
                    nc.sync.dma_start(
                        gp[CQK : CQK + 1, ts(4 * g4 + k, 128)],
                        nmneg[k : k + 1, ds(g4 * 128, 128)],
                    )

            E_tiles = {}

            def emit_sg(jt, grp):
                E = E_tiles[jt]
                psc = pssc.tile([128, EG, JT], F32, name="psc")
                for q in range(EG):
                    ic = grp * EG + q
                    nc.tensor.matmul(
                        psc[:, q, :],
                        fp[:, ts(ic, 128)],
                        gp[:, ts(jt, JT)],
                        start=True,
                        stop=True,
                    )
                nc.scalar.activation(
                    E[:, grp * EG : (grp + 1) * EG, :], psc, AF.Exp
                )

            def emit_o_pass(jt, jc2, inserts, tail):
                """One 128-row o^T accumulation pass; `inserts` is a list of
                8 lists of thunks, one consumed after every 4th i-step."""
                E = E_tiles[jt]
                j0 = jt * JT + jc2 * 128
                ttt = ttrp.tile([128, C], F32, name="ttt")
                nc.sync.dma_start(ttt, ttr_d[ds(j0, 128), :])
                poa = psum_pools["o"].tile([128, 256], F32, tag="poa", name="poa")
                pob = psum_pools["o"].tile([128, 258], F32, tag="pob", name="pob")
                for ic in range(NIC):
                    lhs = E[:, ic, ts(jc2, 128)]
                    nc.tensor.matmul(
                        poa,
                        lhs,
                        vt[:, ic, 0:256],
                        start=(ic == 0),
                        stop=(ic == NIC - 1),
                    )
                    nc.tensor.matmul(
                        pob,
                        lhs,
                        vt[:, ic, 256 : C + 2],
                        start=(ic == 0),
                        stop=(ic == NIC - 1),
                    )
                    if ic % 4 == 3:
                        for thunk in inserts[ic // 4]:
                            thunk()
                zinv = zp.tile([128, 1], F32, name="zinv")
                nc.vector.reciprocal(zinv, pob[:, 256:257])
                nc.vector.tensor_mul(zinv, zinv, gam_sb)
                ob = obp.tile([128, C], F32, name="ob")
                nc.vector.scalar_tensor_tensor(
                    ob[:, 0:256], poa, zinv, ttt[:, 0:256],
                    op0=ALU.mult, op1=ALU.add,
                )
                nc.gpsimd.dma_start(o_d[ds(j0, 128), 0:256], ob[:, 0:256])
                nc.vector.scalar_tensor_tensor(
                    ob[:, 256:C], pob[:, 0:256], zinv, ttt[:, 256:C],
                    op0=ALU.mult, op1=ALU.add,
                )
                nc.gpsimd.dma_start(o_d[ds(j0, 128), 256:C], ob[:, 256:C])
                for thunk in tail:
                    thunk()

            def emit_o_pass0(inserts):
                """Merged jc0+jc1 o-accumulation for tile 0: 4 matmuls per
                i-chunk into 4 PSUM banks, one insert list consumed after
                every 2nd i-step (16 slots). This halves the per-item data
                demand so the pass paces with the t/x DMA stream."""
                E = E_tiles[0]
                acc = []
                for jc2 in range(2):
                    j0 = jc2 * 128
                    ttt = ttrp.tile([128, C], F32, name="ttt")
                    nc.sync.dma_start(ttt, ttr_d[ds(j0, 128), :])
                    poa = psum_pools["o"].tile(
                        [128, 256], F32, tag="poa", name="poa"
                    )
                    pob = psum_pools["o"].tile(
                        [128, 258], F32, tag="pob", name="pob"
                    )
                    acc.append((j0, ttt, poa, pob))
                for ic in range(NIC):
                    for jc2 in range(2):
                        j0, ttt, poa, pob = acc[jc2]
                        lhs = E[:, ic, ts(jc2, 128)]
                        nc.tensor.matmul(
                            poa, lhs, vt[:, ic, 0:256],
                            start=(ic == 0), stop=(ic == NIC - 1),
                        )
                        nc.tensor.matmul(
                            pob, lhs, vt[:, ic, 256 : C + 2],
                            start=(ic == 0), stop=(ic == NIC - 1),
                        )
                    if ic % 2 == 1:
                        for thunk in inserts[ic // 2]:
                            thunk()
                for j0, ttt, poa, pob in acc:
                    zinv = zp.tile([128, 1], F32, name="zinv")
                    nc.vector.reciprocal(zinv, pob[:, 256:257])
                    nc.vector.tensor_mul(zinv, zinv, gam_sb)
                    ob = obp.tile([128, C], F32, name="ob")
                    nc.vector.scalar_tensor_tensor(
                        ob[:, 0:256], poa, zinv, ttt[:, 0:256],
                        op0=ALU.mult, op1=ALU.add,
                    )
                    nc.gpsimd.dma_start(o_d[ds(j0, 128), 0:256], ob[:, 0:256])
                    nc.vector.scalar_tensor_tensor(
                        ob[:, 256:C], pob[:, 0:256], zinv, ttt[:, 256:C],
                        op0=ALU.mult, op1=ALU.add,
                    )
                    nc.gpsimd.dma_start(o_d[ds(j0, 128), 256:C], ob[:, 256:C])

            def sg(jt, grp):
                return lambda: emit_sg(jt, grp)

            def vv(ic):
                return lambda: emit_v(ic)

            def ff(it):
                return lambda: emit_f(it)

            def wa(n):
                return lambda: emit_warm(n)

            # ---- prologue ----
            for jt in range(NJT):
                E_tiles[jt] = epool.tile([128, NIC, JT], BF16, name="E")

            with tc.tile_pool(name="pset", bufs=2, space="PSUM") as pset:
                psum_pools["set"] = pset

                def gg(jt4):
                    def run():
                        emit_g(jt4)
                        for jc in range(4 * jt4, 4 * jt4 + 4):
                            emit_p1(jc)
                        emit_mh(jt4)
                    return run

                # ---- ramp + stream head ----
                emit_warm(44)
                emit_f(0)
                gg(0)()
                emit_sg(0, 0)
                emit_sg(0, 1)
                for ic in range(0, 4):
                    emit_v(ic)

                # ---- tile-0 o-accumulation IS the prologue: it chases the
                # t/x streams, weaving in f, g, v^T, and the score groups of
                # tiles 0 and 1 as their inputs land ----
                with tc.tile_pool(name="pso", bufs=2, space="PSUM") as pso:
                    psum_pools["o"] = pso
                    emit_o_pass0([
                        [wa(1), ff(1), sg(0, 2), gg(1), vv(4), vv(5), sg(2, 0)],
                        [wa(1), sg(0, 3), sg(1, 0), vv(6), vv(7), sg(2, 1)],
                        [wa(1), ff(2), sg(0, 4), sg(1, 1), vv(8), vv(9),
                         sg(2, 2)],
                        [wa(1), sg(0, 5), sg(1, 2), vv(10), vv(11), sg(2, 3)],
                        [wa(1), ff(3), sg(0, 6), sg(1, 3), vv(12), vv(13),
                         sg(2, 4)],
                        [wa(1), sg(0, 7), sg(1, 4), vv(14), vv(15), sg(2, 5)],
                        [wa(1), ff(4), sg(0, 8), sg(1, 5), vv(16), vv(17),
                         sg(2, 6)],
                        [wa(1), sg(0, 9), sg(1, 6), vv(18), vv(19), sg(2, 7)],
                        [wa(1), gg(2), ff(5), sg(0, 10), vv(20), vv(21),
                         sg(2, 8)],
                        [wa(1), sg(0, 11), sg(1, 7), vv(22), vv(23), sg(2, 9)],
                        [wa(1), ff(6), sg(0, 12), sg(1, 8), vv(24), vv(25),
                         sg(2, 10)],
                        [wa(1), sg(0, 13), sg(1, 9), vv(26), vv(27),
                         sg(2, 11)],
                        [wa(1), ff(7), sg(0, 14), sg(1, 10), vv(28), vv(29),
                         sg(2, 12)],
                        [wa(1), sg(0, 15), sg(1, 11), vv(30), vv(31),
                         sg(2, 13)],
                        [wa(1), gg(3), sg(1, 12), sg(1, 13), sg(2, 14)],
                        [wa(1), sg(1, 14), sg(1, 15), sg(2, 15)],
                    ])

                    # ---- steady state (score tiles woven two ahead) ----
                    for jt in range(1, NJT):
                        for jc2 in range(JT // 128):
                            ins = [[] for _ in range(8)]
                            if jt + 2 < NJT:
                                for k in range(8):
                                    ins[k] = [sg(jt + 2, jc2 * 8 + k)]
                            emit_o_pass(jt, jc2, ins, [])

    nc.compile()
    return nc


_NC_CACHE = None


def _get_nc():
    global _NC_CACHE
    if _NC_CACHE is None:
        _NC_CACHE = _build()
    return _NC_CACHE


def make_in_maps(origin_out, target_in, Wq, Wk, Wv, gamma):
    x_b = np.ascontiguousarray(
        np.asarray(origin_out, dtype=np.float32).reshape(B, C, N)
    )
    t_b = np.ascontiguousarray(
        np.asarray(target_in, dtype=np.float32).reshape(B, C, N)
    )
    wqt = np.ascontiguousarray(np.asarray(Wq, dtype=np.float32).T.astype(np.float16))
    wkt = np.ascontiguousarray(np.asarray(Wk, dtype=np.float32).T.astype(np.float16))
    wvt = np.ascontiguousarray(np.asarray(Wv, dtype=np.float32).T.astype(np.float16))
    gam = np.full((128, 1), np.asarray(gamma, dtype=np.float32).reshape(-1)[0],
                  dtype=np.float32)
    in_maps = []
    for core in range(NCORES):
        b, half = core // 2, core % 2
        j0 = half * NJ
        # permute the i axis so this core's j-shard columns come first
        # (i is contracted, softmax over i is permutation-invariant)
        if half == 0:
            xp, tp = x_b[b], t_b[b]
        else:
            xp = np.concatenate([x_b[b][:, NJ:], x_b[b][:, :NJ]], axis=1)
            tp = np.concatenate([t_b[b][:, NJ:], t_b[b][:, :NJ]], axis=1)
        xg = np.ascontiguousarray(
            xp.reshape(NCC, 128, NIC // 4, 4, 128)
            .transpose(2, 1, 0, 3, 4)
            .reshape(NIC // 4, 128, NCC * 4 * 128)
        )
        in_maps.append(
            {
                "x": np.ascontiguousarray(xg[:4].astype(np.float16)),
                "xb": np.ascontiguousarray(xg[4:].astype(ml_dtypes.bfloat16)),
                "t": np.ascontiguousarray(tp.astype(np.float16)),
                "ttr": np.ascontiguousarray(t_b[b][:, j0 : j0 + NJ].T),
                "wqt": wqt,
                "wkt": wkt,
                "wvt": wvt,
                "gam": gam,
            }
        )
    return in_maps


def run_cores(in_maps, **kwargs):
    nc = _get_nc()
    return run_bass_kernel_spmd(nc, in_maps, core_ids=list(range(NCORES)), **kwargs)


def assemble(results):
    o = np.empty((B, C, N), dtype=np.float32)
    for core in range(NCORES):
        b, half = core // 2, core % 2
        j0 = half * NJ
        o[b][:, j0 : j0 + NJ] = results[core]["o"].T
    return o.reshape(B, C, H, W)


def kernel(origin_out, target_in, Wq, Wk, Wv, gamma):
    in_maps = make_in_maps(origin_out, target_in, Wq, Wk, Wv, gamma)
    res = run_cores(in_maps)
    return assemble(res.results)


# revision 31
# speedup vs baseline: 1.2239x; 1.2239x over previous
"""Trainium2 Bass kernel for AttentionLateralOp.

Reference computation (per batch b):
    x = origin_out[b].reshape(C, N)      # keys/values source
    t = target_in[b].reshape(C, N)       # queries source + residual
    f = Wq @ t          [CQK, N]
    g = Wk @ x          [CQK, N]
    v = Wv @ x          [C, N]
    scores = f^T @ g    [N, N]
    beta = softmax(scores, axis=0)       # over i (rows)
    o = gamma * v @ beta + t

Sharding: 8 cores = (batch b = core//2) x (half of the j/output axis =
core%2). Each core computes the full f and v^T for its batch, and the
j-shard of g / scores / output.

Pipeline layout (v3): one fused stream ordered by data arrival.
 - DMA is striped round-robin across the three hardware queues
   (sync/scalar/gpsimd) in 0.5MB units sorted by deadline, so each
   tensor lands roughly when its consumer needs it.
 - The in-order PE queue is emitted to chase the stream: f chunks as t
   arrives, g + subsampled row-max per x j-shard group, v^T chunks as
   soon as Wv and x land, then score-tile 0 group-by-group (group g
   needs only f chunk g), with the o-accumulation of tile 0 chasing the
   exp of tile 0 while the tail of f / v^T / score-tile-1 is woven in.
 - Steady state: o-accumulation for tile k with the score matmuls of
   tile k+1 woven in every 8 i-steps. PSUM: 2x2-bank score buffers +
   2x2-bank output accumulators = 8 banks.
 - E and v^T are held in bf16 (halves SBUF, same PE rate); exp runs on
   Scalar in 1024-element groups chasing the score matmuls.

Softmax-over-the-contraction-axis trick: append a ones row to f and a
(-mhat_j) row to g, so the PE emits max-subtracted logits directly into
PSUM; Z_j comes from a ones column appended to v^T; the final gamma/Z_j
scaling and +t residual are per-partition ops in the transposed [j, c]
output orientation (output is transposed back on the host).
"""

import os
import sys

for _p in ("/opt/trn_rl_repo", "/root/.axon_site/_ro/trn_rl_repo"):
    if os.path.isdir(_p):
        sys.path.insert(0, _p)
        break

import ml_dtypes
import numpy as np

import concourse.bass as bass  # noqa: F401  (bass types via bacc)
import concourse.tile as tile
from concourse import bacc, mybir
from concourse.bass import ds, ts
from concourse.bass_utils import run_bass_kernel_spmd
from concourse.masks import make_identity

F32 = mybir.dt.float32
F32R = mybir.dt.float32r
F16 = mybir.dt.float16
BF16 = mybir.dt.bfloat16
AF = mybir.ActivationFunctionType
ALU = mybir.AluOpType
AX = mybir.AxisListType

B, C, H, W = 4, 512, 64, 64
N = H * W            # 4096
CQK = C // 8         # 64
NCORES = 8
NJ = B * N // NCORES  # 2048 columns of the j axis per core
JT = 256             # j-tile width of a score tile
NJT = NJ // JT       # 8 score tiles
NIC = N // 128       # 32 i-chunks
NCC = C // 128       # 4 contraction chunks over C
EG = 2               # i-chunks per exp group (psc tile = 1 PSUM bank)
NEG = NIC // EG      # 8 exp groups per score tile


def _build():
    nc = bacc.Bacc(None, target_bir_lowering=False)

    x_d = nc.dram_tensor(
        "x", [NIC // 8, 128, NCC * 4 * 128], F16, kind="ExternalInput"
    )
    xb_d = nc.dram_tensor(
        "xb", [NIC // 8, 128, NCC * 4 * 128], BF16, kind="ExternalInput"
    )
    t_d = nc.dram_tensor("t", [C, N], F16, kind="ExternalInput")
    ttr_d = nc.dram_tensor("ttr", [NJ, C], F32, kind="ExternalInput")
    wqt_d = nc.dram_tensor("wqt", [C, CQK], F16, kind="ExternalInput")
    wkt_d = nc.dram_tensor("wkt", [C, CQK], F16, kind="ExternalInput")
    wvt_d = nc.dram_tensor("wvt", [C, C], F16, kind="ExternalInput")
    gam_d = nc.dram_tensor("gam", [128, 1], F32, kind="ExternalInput")
    o_d = nc.dram_tensor("o", [NJ, C], F32, kind="ExternalOutput")

    with tile.TileContext(nc) as tc:
        with (
            tc.tile_pool(name="persist", bufs=1) as persist,
            tc.tile_pool(name="wpool", bufs=1) as wpool,
            tc.tile_pool(name="tstream", bufs=8) as tstream,
            tc.tile_pool(name="xfpool", bufs=4) as xfpool,
            tc.tile_pool(name="xfbpool", bufs=4) as xfbpool,
            tc.tile_pool(name="epool", bufs=3) as epool,
            tc.tile_pool(name="ttrp", bufs=2) as ttrp,
            tc.tile_pool(name="obp", bufs=2) as obp,
            tc.tile_pool(name="zp", bufs=2) as zp,
            tc.tile_pool(name="pssc", bufs=2, space="PSUM") as pssc,
        ):
            # ---- persistent SBUF ----
            vt = persist.tile([128, NIC, C + 2], BF16)
            fp = persist.tile([CQK + 1, N], F32R)
            gp = persist.tile([CQK + 1, NJ], F32R)
            ident = persist.tile([128, 128], F32)
            identb = persist.tile([128, 128], BF16)
            mall = persist.tile([128, 16], BF16)
            nmneg = persist.tile([4, 16 * 128], F32R)
            gam_sb = persist.tile([128, 1], F32)

            wqt_sb = wpool.tile([128, NCC, CQK], F16)
            wkt_sb = wpool.tile([128, NCC, CQK], F16)
            wvt_sb = wpool.tile([128, NCC, C], F16)
            wvb_sb = wpool.tile([128, NCC, C], BF16)
            warm_sb = persist.tile([128, 256], F32R)

            # ---- DMA: build the deadline-ordered unit list, stripe RR ----
            t_tiles = {
                it: tstream.tile([128, NCC, 512], F16, name="tt")
                for it in range(N // 512)
            }
            xf_tiles = {
                g4: xfpool.tile([128, NCC, 4, 128], F16, name="xf")
                for g4 in range(4)
            }
            xfb_tiles = {
                g4: xfbpool.tile([128, NCC, 4, 128], BF16, name="xfb")
                for g4 in range(4, 8)
            }

            def u_t(it):
                return lambda eng: eng.dma_start(
                    t_tiles[it],
                    t_d[:, ts(it, 512)].rearrange("(cc p) n -> p cc n", cc=NCC),
                )

            def u_x(g4):
                return lambda eng: eng.dma_start(xf_tiles[g4], x_d[g4])

            def u_xb(g4):  # one bf16 x group (v-only tail), 0.5MB
                return lambda eng: eng.dma_start(
                    xfb_tiles[g4], xb_d[g4 - 4]
                )

            def u_wq(eng):
                eng.dma_start(
                    wqt_sb, wqt_d.rearrange("(cc p) k -> p cc k", cc=NCC)
                )

            def u_wk(eng):
                eng.dma_start(
                    wkt_sb, wkt_d.rearrange("(cc p) k -> p cc k", cc=NCC)
                )

            def u_wv(eng):
                eng.dma_start(
                    wvt_sb, wvt_d.rearrange("(cc p) e -> p cc e", cc=NCC)
                )

            def u_gam(eng):
                eng.dma_start(gam_sb, gam_d[:])

            units = [u_wq, u_wk, u_t(0), u_wv, u_x(0), u_t(1), u_x(1),
                     u_t(2), u_x(2), u_t(3), u_x(3), u_t(4), u_xb(4),
                     u_t(5), u_xb(5), u_t(6), u_xb(6), u_t(7), u_xb(7),
                     u_gam]
            engs = [nc.sync, nc.scalar, nc.gpsimd]
            # Latin-square dealing: consecutive units AND units 3 apart land
            # on different queues (the unit list has period-3 patterns that
            # would otherwise pin both halves of one tensor to one queue)
            _deal = [0, 1, 2, 1, 2, 0, 2, 0, 1]
            for i, u in enumerate(units):
                u(engs[_deal[i % 9]])

            # constants; warm_sb feeds the HAM warm-up matmuls and must be
            # ready immediately, so its memset rides on vector (no DMA queue)
            nc.vector.memset(warm_sb.bitcast(F32), 0.0)
            make_identity(nc, ident)
            nc.vector.tensor_copy(identb, ident)
            nc.gpsimd.memset(fp[CQK : CQK + 1, :].bitcast(F32), 1.0)
            nc.gpsimd.memset(vt[:, :, C : C + 2], 1.0)
            # bf16 copy of Wv^T for the bf16 v^T tail
            nc.vector.tensor_copy(wvb_sb, wvt_sb)

            # ---- compute emission helpers ----
            psum_pools = {}

            def emit_f(it):
                pf = psum_pools["set"].tile([CQK, 512], F32, tag="ps", name="pf")
                for cc in range(NCC):
                    nc.tensor.matmul(
                        pf,
                        wqt_sb[:, cc, :],
                        t_tiles[it][:, cc, :],
                        start=(cc == 0),
                        stop=(cc == NCC - 1),
                    )
                nc.vector.tensor_copy(fp[0:CQK, ts(it, 512)], pf)

            def emit_v(ic):
                pv = psum_pools["set"].tile([128, C], F32, tag="ps", name="pv")
                g4 = ic // 4
                lhs_t = xf_tiles[g4] if g4 < 4 else xfb_tiles[g4]
                rhs_w = wvt_sb if g4 < 4 else wvb_sb
                for cc in range(NCC):
                    nc.tensor.matmul(
                        pv,
                        lhs_t[:, cc, ic % 4, :],
                        rhs_w[:, cc, :],
                        start=(cc == 0),
                        stop=(cc == NCC - 1),
                    )
                nc.vector.tensor_copy(vt[:, ic, 0:C], pv)

            def emit_warm(n):
                # dummy matmuls: keep the PE busy through DMA waits so the
                # HAM clock gate reaches (and holds) 2.4 GHz from the start.
                # Four accumulating matmuls per PSUM tile -- a single matmul
                # per tile paces at the slot-reuse semaphore latency (~35%
                # duty), which HAM reads as idle.
                for _ in range(n):
                    pw = psum_pools["set"].tile(
                        [128, 256], F32, tag="ps", name="pw"
                    )
                    nc.tensor.matmul(
                        pw, warm_sb[:, 0:128], warm_sb, start=True, stop=True
                    )

            def emit_g(jt4):
                pg = psum_pools["set"].tile([CQK, 512], F32, tag="ps", name="pg")
                for cc in range(NCC):
                    nc.tensor.matmul(
                        pg,
                        wkt_sb[:, cc, :],
                        xf_tiles[jt4][:, cc, :, :],
                        start=(cc == 0),
                        stop=(cc == NCC - 1),
                    )
                nc.vector.tensor_copy(gp[0:CQK, ts(jt4, 512)], pg)

            def emit_p1(jc):
                ps1 = pssc.tile([128, 256], F32, name="psc")
                nc.tensor.matmul(
                    ps1,
                    gp[0:CQK, ts(jc, 128)],
                    fp[0:CQK, 0:256],
                    start=True,
                    stop=True,
                )
                nc.vector.reduce_max(mall[:, jc : jc + 1], ps1, axis=AX.X)

            def emit_mh(g4):
                pmt = psum_pools["set"].tile([16, 128], F32, tag="ps", name="pmt")
                nc.tensor.matmul(
                    pmt[0:4, :], mall[:, 4 * g4 : 4 * g4 + 4], ident,
                    start=True, stop=True,
                )
                nc.scalar.mul(
                    nmneg[0:4, ds(g4 * 128, 128)], pmt[0:4, :], -1.0
                )
                for k in range(4):
                    nc.sync.dma_start(
                        gp[CQK : CQK + 1, ts(4 * g4 + k, 128)],
                        nmneg[k : k + 1, ds(g4 * 128, 128)],
                    )

            E_tiles = {}

            def emit_sg(jt, grp):
                E = E_tiles[jt]
                psc = pssc.tile([128, EG, JT], F32, name="psc")
                for q in range(EG):
                    ic = grp * EG + q
                    nc.tensor.matmul(
                        psc[:, q, :],
                        fp[:, ts(ic, 128)],
                        gp[:, ts(jt, JT)],
                        start=True,
                        stop=True,
                    )
                nc.scalar.activation(
                    E[:, grp * EG : (grp + 1) * EG, :], psc, AF.Exp
                )

            def emit_o_pass(jt, jc2, inserts, tail):
                """One 128-row o^T accumulation pass; `inserts` is a list of
                8 lists of thunks, one consumed after every 4th i-step."""
                E = E_tiles[jt]
                j0 = jt * JT + jc2 * 128
                ttt = ttrp.tile([128, C], F32, name="ttt")
                nc.sync.dma_start(ttt, ttr_d[ds(j0, 128), :])
                poa = psum_pools["o"].tile([128, 256], F32, tag="poa", name="poa")
                pob = psum_pools["o"].tile([128, 258], F32, tag="pob", name="pob")
                for ic in range(NIC):
                    lhs = E[:, ic, ts(jc2, 128)]
                    nc.tensor.matmul(
                        poa,
                        lhs,
                        vt[:, ic, 0:256],
                        start=(ic == 0),
                        stop=(ic == NIC - 1),
                    )
                    nc.tensor.matmul(
                        pob,
                        lhs,
                        vt[:, ic, 256 : C + 2],
                        start=(ic == 0),
                        stop=(ic == NIC - 1),
                    )
                    if ic % 4 == 3:
                        for thunk in inserts[ic // 4]:
                            thunk()
                zinv = zp.tile([128, 1], F32, name="zinv")
                nc.vector.reciprocal(zinv, pob[:, 256:257])
                nc.vector.tensor_mul(zinv, zinv, gam_sb)
                ob = obp.tile([128, C], F32, name="ob")
                nc.vector.scalar_tensor_tensor(
                    ob[:, 0:256], poa, zinv, ttt[:, 0:256],
                    op0=ALU.mult, op1=ALU.add,
                )
                nc.sync.dma_start(o_d[ds(j0, 128), 0:256], ob[:, 0:256])
                nc.vector.scalar_tensor_tensor(
                    ob[:, 256:C], pob[:, 0:256], zinv, ttt[:, 256:C],
                    op0=ALU.mult, op1=ALU.add,
                )
                nc.sync.dma_start(o_d[ds(j0, 128), 256:C], ob[:, 256:C])
                for thunk in tail:
                    thunk()

            def emit_o_pass0(inserts):
                """Merged jc0+jc1 o-accumulation for tile 0: 4 matmuls per
                i-chunk into 4 PSUM banks, one insert list consumed after
                every 2nd i-step (16 slots). This halves the per-item data
                demand so the pass paces with the t/x DMA stream."""
                E = E_tiles[0]
                acc = []
                for jc2 in range(2):
                    j0 = jc2 * 128
                    ttt = ttrp.tile([128, C], F32, name="ttt")
                    nc.sync.dma_start(ttt, ttr_d[ds(j0, 128), :])
                    poa = psum_pools["o"].tile(
                        [128, 256], F32, tag="poa", name="poa"
                    )
                    pob = psum_pools["o"].tile(
                        [128, 258], F32, tag="pob", name="pob"
                    )
                    acc.append((j0, ttt, poa, pob))
                for ic in range(NIC):
                    for jc2 in range(2):
                        j0, ttt, poa, pob = acc[jc2]
                        lhs = E[:, ic, ts(jc2, 128)]
                        nc.tensor.matmul(
                            poa, lhs, vt[:, ic, 0:256],
                            start=(ic == 0), stop=(ic == NIC - 1),
                        )
                        nc.tensor.matmul(
                            pob, lhs, vt[:, ic, 256 : C + 2],
                            start=(ic == 0), stop=(ic == NIC - 1),
                        )
                    if ic % 2 == 1:
                        for thunk in inserts[ic // 2]:
                            thunk()
                for j0, ttt, poa, pob in acc:
                    zinv = zp.tile([128, 1], F32, name="zinv")
                    nc.vector.reciprocal(zinv, pob[:, 256:257])
                    nc.vector.tensor_mul(zinv, zinv, gam_sb)
                    ob = obp.tile([128, C], F32, name="ob")
                    nc.vector.scalar_tensor_tensor(
                        ob[:, 0:256], poa, zinv, ttt[:, 0:256],
                        op0=ALU.mult, op1=ALU.add,
                    )
                    nc.sync.dma_start(o_d[ds(j0, 128), 0:256], ob[:, 0:256])
                    nc.vector.scalar_tensor_tensor(
                        ob[:, 256:C], pob[:, 0:256], zinv, ttt[:, 256:C],
                        op0=ALU.mult, op1=ALU.add,
                    )
                    nc.sync.dma_start(o_d[ds(j0, 128), 256:C], ob[:, 256:C])

            def sg(jt, grp):
                return lambda: emit_sg(jt, grp)

            def vv(ic):
                return lambda: emit_v(ic)

            def ff(it):
                return lambda: emit_f(it)

            def wa(n):
                return lambda: emit_warm(n)

            # ---- prologue ----
            for jt in range(NJT):
                E_tiles[jt] = epool.tile([128, NIC, JT], BF16, name="E")

            with tc.tile_pool(name="pset", bufs=2, space="PSUM") as pset:
                psum_pools["set"] = pset

                def gg(jt4):
                    def run():
                        emit_g(jt4)
                        for jc in range(4 * jt4, 4 * jt4 + 4):
                            emit_p1(jc)
                        emit_mh(jt4)
                    return run

                # ---- ramp + stream head ----
                emit_warm(44)
                emit_f(0)
                gg(0)()
                emit_sg(0, 0)
                emit_sg(0, 1)
                for ic in range(0, 4):
                    emit_v(ic)

                # ---- tile-0 o-accumulation IS the prologue: it chases the
                # t/x streams, weaving in f, g, v^T, and the score groups of
                # tiles 0 and 1 as their inputs land ----
                with tc.tile_pool(name="pso", bufs=2, space="PSUM") as pso:
                    psum_pools["o"] = pso
                    emit_o_pass0([
                        [wa(1), ff(1), sg(0, 2), gg(1), vv(4), vv(5), sg(2, 0)],
                        [wa(1), sg(0, 3), sg(1, 0), vv(6), vv(7), sg(2, 1)],
                        [wa(1), ff(2), sg(0, 4), sg(1, 1), vv(8), vv(9),
                         sg(2, 2)],
                        [wa(1), sg(0, 5), sg(1, 2), vv(10), vv(11), sg(2, 3)],
                        [wa(1), ff(3), sg(0, 6), sg(1, 3), vv(12), vv(13),
                         sg(2, 4)],
                        [wa(1), sg(0, 7), sg(1, 4), vv(14), vv(15), sg(2, 5)],
                        [wa(1), ff(4), sg(0, 8), sg(1, 5), vv(16), vv(17),
                         sg(2, 6)],
                        [wa(1), sg(0, 9), sg(1, 6), vv(18), vv(19), sg(2, 7)],
                        [wa(1), gg(2), ff(5), sg(0, 10), vv(20), vv(21),
                         sg(2, 8)],
                        [wa(1), sg(0, 11), sg(1, 7), vv(22), vv(23), sg(2, 9)],
                        [wa(1), ff(6), sg(0, 12), sg(1, 8), vv(24), vv(25),
                         sg(2, 10)],
                        [wa(1), sg(0, 13), sg(1, 9), vv(26), vv(27),
                         sg(2, 11)],
                        [wa(1), ff(7), sg(0, 14), sg(1, 10), vv(28), vv(29),
                         sg(2, 12)],
                        [wa(1), sg(0, 15), sg(1, 11), vv(30), vv(31),
                         sg(2, 13)],
                        [wa(1), gg(3), sg(1, 12), sg(1, 13), sg(2, 14)],
                        [wa(1), sg(1, 14), sg(1, 15), sg(2, 15)],
                    ])

                    # ---- steady state (score tiles woven two ahead) ----
                    for jt in range(1, NJT):
                        for jc2 in range(JT // 128):
                            ins = [[] for _ in range(8)]
                            if jt + 2 < NJT:
                                for k in range(8):
                                    ins[k] = [sg(jt + 2, jc2 * 8 + k)]
                            emit_o_pass(jt, jc2, ins, [])

    nc.compile()
    return nc


_NC_CACHE = None


def _get_nc():
    global _NC_CACHE
    if _NC_CACHE is None:
        _NC_CACHE = _build()
    return _NC_CACHE


def make_in_maps(origin_out, target_in, Wq, Wk, Wv, gamma):
    x_b = np.ascontiguousarray(
        np.asarray(origin_out, dtype=np.float32).reshape(B, C, N)
    )
    t_b = np.ascontiguousarray(
        np.asarray(target_in, dtype=np.float32).reshape(B, C, N)
    )
    wqt = np.ascontiguousarray(np.asarray(Wq, dtype=np.float32).T.astype(np.float16))
    wkt = np.ascontiguousarray(np.asarray(Wk, dtype=np.float32).T.astype(np.float16))
    wvt = np.ascontiguousarray(np.asarray(Wv, dtype=np.float32).T.astype(np.float16))
    gam = np.full((128, 1), np.asarray(gamma, dtype=np.float32).reshape(-1)[0],
                  dtype=np.float32)
    in_maps = []
    for core in range(NCORES):
        b, half = core // 2, core % 2
        j0 = half * NJ
        # permute the i axis so this core's j-shard columns come first
        # (i is contracted, softmax over i is permutation-invariant)
        if half == 0:
            xp, tp = x_b[b], t_b[b]
        else:
            xp = np.concatenate([x_b[b][:, NJ:], x_b[b][:, :NJ]], axis=1)
            tp = np.concatenate([t_b[b][:, NJ:], t_b[b][:, :NJ]], axis=1)
        xg = np.ascontiguousarray(
            xp.reshape(NCC, 128, NIC // 4, 4, 128)
            .transpose(2, 1, 0, 3, 4)
            .reshape(NIC // 4, 128, NCC * 4 * 128)
        )
        in_maps.append(
            {
                "x": np.ascontiguousarray(xg[:4].astype(np.float16)),
                "xb": np.ascontiguousarray(xg[4:].astype(ml_dtypes.bfloat16)),
                "t": np.ascontiguousarray(tp.astype(np.float16)),
                "ttr": np.ascontiguousarray(t_b[b][:, j0 : j0 + NJ].T),
                "wqt": wqt,
                "wkt": wkt,
                "wvt": wvt,
                "gam": gam,
            }
        )
    return in_maps


def run_cores(in_maps, **kwargs):
    nc = _get_nc()
    return run_bass_kernel_spmd(nc, in_maps, core_ids=list(range(NCORES)), **kwargs)


def assemble(results):
    o = np.empty((B, C, N), dtype=np.float32)
    for core in range(NCORES):
        b, half = core // 2, core % 2
        j0 = half * NJ
        o[b][:, j0 : j0 + NJ] = results[core]["o"].T
    return o.reshape(B, C, H, W)


def kernel(origin_out, target_in, Wq, Wk, Wv, gamma):
    in_maps = make_in_maps(origin_out, target_in, Wq, Wk, Wv, gamma)
    res = run_cores(in_maps)
    return assemble(res.results)
